# revision 10
# baseline (speedup 1.0000x reference)
"""HalfKA NNUE forward pass on 8 Trainium2 NeuronCores.

Network (fp32 reference):
    h1  = relu(x @ W1.T + b1)     x:[2048, 98304] sparse 0/1, W1:[256, 98304]
    h2  = relu(h1 @ W2.T + b2)    W2:[32, 256]
    out = h2 @ Wout.T + bout      Wout:[1, 32]  -> [2048, 1]

Strategy: tensor-parallel over the fc1 contraction (input_dim). Each of the 8
cores reads its own 12288-column slice of x (the dominant 100 MB/core stream)
plus a 12288-row slice of W1, accumulates a partial h1 [256, 2048] on the PE
array, and the partials are summed with an on-device AllReduce. fc2/fc3 are
tiny and computed (redundantly) on every core in fp32.

fc1 runs as two bf16 passes (W1 split into bf16 hi + lo parts on the host,
accumulated in fp32 PSUM) which is fp32-accurate to ~1e-6 while streaming the
PE at 1 cycle/row instead of fp32's 4. x is 0/1 so its bf16 cast is exact.

The batch is processed in 4 chunks of 512 so each chunk's AllReduce overlaps
the next chunk's fc1 matmuls.
"""

import os
import sys

sys.path.insert(0, "/opt/trn_rl_repo")

from contextlib import ExitStack

import numpy as np
import ml_dtypes

import concourse.bass as bass
import concourse.tile as tile
from concourse import bacc, mybir
from concourse.bass_utils import run_bass_kernel_spmd

f32 = mybir.dt.float32
bf16 = mybir.dt.bfloat16

N_CORES = 8
B = 2048
IN_DIM = 98304
H1 = 256
H2 = 32

P = 128
KSH = IN_DIM // N_CORES          # 12288 contraction dims per core
KT = KSH // P                    # 96 k-tiles per core
SUP = 8                          # k-tiles per x DMA super-tile (16 KB/partition)
NSUP = KT // SUP                 # 12
CHUNK = 512                      # batch columns per chunk (one PSUM bank)
NCH = B // CHUNK                 # 4
M_T = H1 // P                    # 2 h1 partition-tiles

_CACHED = {}


def _build_program():
    nc = bacc.Bacc(
        "TRN2",
        target_bir_lowering=False,
        debug=False,
        num_devices=N_CORES,
    )

    xt = nc.dram_tensor("xt", [NCH, P, KT // SUP, SUP, CHUNK], f32, kind="ExternalInput")
    w1hi = nc.dram_tensor("w1hi", [P, KT, H1], bf16, kind="ExternalInput")
    w1lo = nc.dram_tensor("w1lo", [P, KT, H1], bf16, kind="ExternalInput")
    b1 = nc.dram_tensor("b1", [P, M_T], f32, kind="ExternalInput")
    w2t = nc.dram_tensor("w2t", [P, M_T, H2], f32, kind="ExternalInput")
    b2 = nc.dram_tensor("b2", [H2, 1], f32, kind="ExternalInput")
    # [Wout.T; bout] stacked: fc3 computes Wout @ h2 + bout via a ones-row in h2
    woutt = nc.dram_tensor("woutt", [H2 + 1, 1], f32, kind="ExternalInput")
    out = nc.dram_tensor("out", [NCH, CHUNK], f32, kind="ExternalOutput")

    with tile.TileContext(nc) as tc:
        with ExitStack() as ctx:
            const = ctx.enter_context(tc.tile_pool(name="const", bufs=1))
            xfp = ctx.enter_context(tc.tile_pool(name="xf", bufs=2))
            xbp = ctx.enter_context(tc.tile_pool(name="xb", bufs=2))
            drp = ctx.enter_context(tc.tile_pool(name="drain", bufs=4))
            h1rp = ctx.enter_context(tc.tile_pool(name="h1r", bufs=4))
            h1ap = ctx.enter_context(tc.tile_pool(name="h1a", bufs=4))
            smp = ctx.enter_context(tc.tile_pool(name="small", bufs=4))
            ps1 = ctx.enter_context(tc.tile_pool(name="ps1", bufs=2, space="PSUM"))
            ps2 = ctx.enter_context(tc.tile_pool(name="ps2", bufs=2, space="PSUM"))
            ps3 = ctx.enter_context(tc.tile_pool(name="ps3", bufs=2, space="PSUM"))
            dram = ctx.enter_context(tc.tile_pool(name="dram", bufs=2 * NCH, space="DRAM"))

            # ---- resident constants ----
            w1hi_s = const.tile([P, KT, H1], bf16)
            w1lo_s = const.tile([P, KT, H1], bf16)
            nc.sync.dma_start(w1hi_s[:], w1hi.ap())
            nc.sync.dma_start(w1lo_s[:], w1lo.ap())
            b1_s = const.tile([P, M_T], f32)
            nc.sync.dma_start(b1_s[:], b1.ap())
            w2t_s = const.tile([P, M_T, H2], f32)
            nc.sync.dma_start(w2t_s[:], w2t.ap())
            b2_s = const.tile([H2, 1], f32)
            nc.sync.dma_start(b2_s[:], b2.ap())
            woutt_s = const.tile([H2 + 1, 1], f32)
            nc.sync.dma_start(woutt_s[:], woutt.ap())

            for j in range(NCH):
                # ---- fc1: partial h1[256, 512] for this chunk ----
                psum_m = [
                    ps1.tile([P, CHUNK], f32, tag=f"ps1_{m}", name=f"ps1m{m}_c{j}")
                    for m in range(M_T)
                ]
                for s in range(NSUP):
                    xf = xfp.tile([P, SUP, CHUNK], f32)
                    nc.sync.dma_start(xf[:], xt.ap()[j, :, s, :, :])
                    xb = xbp.tile([P, SUP, CHUNK], bf16)
                    nc.vector.tensor_copy(xb[:], xf[:])
                    for tt in range(SUP):
                        t = s * SUP + tt
                        for m in range(M_T):
                            nc.tensor.matmul(
                                psum_m[m][:],
                                w1hi_s[:, t, m * P:(m + 1) * P],
                                xb[:, tt, :],
                                start=(t == 0),
                                stop=False,
                            )
                            nc.tensor.matmul(
                                psum_m[m][:],
                                w1lo_s[:, t, m * P:(m + 1) * P],
                                xb[:, tt, :],
                                start=False,
                                stop=(t == KT - 1),
                            )

                # ---- AllReduce the partial across the 8 cores ----
                cc_in = dram.tile([H1, CHUNK], f32, tag="cc_in")
                cc_out = dram.tile([H1, CHUNK], f32, tag="cc_out")
                for m in range(M_T):
                    dr = drp.tile([P, CHUNK], f32)
                    nc.vector.tensor_copy(dr[:], psum_m[m][:])
                    nc.sync.dma_start(cc_in[m * P:(m + 1) * P, :], dr[:])
                nc.gpsimd.collective_compute(
                    "AllReduce",
                    mybir.AluOpType.add,
                    replica_groups=[list(range(N_CORES))],
                    ins=[cc_in.opt()],
                    outs=[cc_out.opt()],
                )

                # ---- bias + relu -> h1, then fc2/fc3 in fp32 ----
                h1a = []
                for m in range(M_T):
                    h1r = h1rp.tile([P, CHUNK], f32)
                    nc.sync.dma_start(h1r[:], cc_out[m * P:(m + 1) * P, :])
                    act = h1ap.tile([P, CHUNK], f32)
                    nc.scalar.activation(
                        act[:], h1r[:],
                        mybir.ActivationFunctionType.Relu,
                        bias=b1_s[:, m:m + 1],
                    )
                    h1a.append(act)

                p2 = ps2.tile([H2, CHUNK], f32)
                for m in range(M_T):
                    nc.tensor.matmul(
                        p2[:], w2t_s[:, m, :], h1a[m][:],
                        start=(m == 0), stop=(m == M_T - 1),
                    )
                h2t = smp.tile([H2 + 1, CHUNK], f32, tag="h2")
                nc.scalar.activation(
                    h2t[0:H2, :], p2[:],
                    mybir.ActivationFunctionType.Relu,
                    bias=b2_s[:],
                )
                nc.vector.memset(h2t[H2:H2 + 1, :], 1.0)

                p3 = ps3.tile([1, CHUNK], f32)
                nc.tensor.matmul(p3[:], woutt_s[:], h2t[:], start=True, stop=True)
                ot = smp.tile([1, CHUNK], f32, tag="ot")
                nc.vector.tensor_copy(ot[:], p3[:])
                nc.sync.dma_start(out.ap()[j, :], ot[:])

    nc.compile()
    return nc


def get_program():
    if "nc" not in _CACHED:
        _CACHED["nc"] = _build_program()
    return _CACHED["nc"]


def _prep_inputs(x, W1, b1, W2, b2, Wout, bout):
    """Shard + lay out host-side into DMA-friendly per-core tensors."""
    bf = ml_dtypes.bfloat16

    # x: [2048, 98304] -> xT [98304, 2048] -> per core [NCH, P, NSUP, SUP, CHUNK]
    xT = np.ascontiguousarray(x.T)  # [98304, 2048]
    # x6[c] axes: [s, t, p, j, n]; device wants [j, p, s, t, n]
    x6 = xT.reshape(N_CORES, NSUP, SUP, P, NCH, CHUNK)
    in_maps = [
        {"xt": np.ascontiguousarray(x6[c].transpose(3, 2, 0, 1, 4))}
        for c in range(N_CORES)
    ]

    w1T = np.ascontiguousarray(W1.T)  # [98304, 256]
    b1_h = np.ascontiguousarray(b1.reshape(M_T, P).T)            # [P, M_T]
    w2t_h = np.ascontiguousarray(W2.T.reshape(M_T, P, H2).transpose(1, 0, 2))  # [P, M_T, H2]
    b2_h = np.ascontiguousarray(b2.reshape(H2, 1))
    woutt_h = np.concatenate(
        [Wout.T, bout.reshape(1, 1)], axis=0
    ).astype(np.float32)                                         # [H2+1, 1]

    for c in range(N_CORES):
        w1T_c = w1T[c * KSH:(c + 1) * KSH]                       # [12288, 256]
        hi = w1T_c.astype(bf)
        lo = (w1T_c - hi.astype(np.float32)).astype(bf)
        # [KSH, H1] -> [P, KT, H1]: row (t*P + p) -> [p, t]
        hi = np.ascontiguousarray(hi.reshape(KT, P, H1).transpose(1, 0, 2))
        lo = np.ascontiguousarray(lo.reshape(KT, P, H1).transpose(1, 0, 2))
        in_maps[c].update({
            "w1hi": hi,
            "w1lo": lo,
            "b1": b1_h,
            "w2t": w2t_h,
            "b2": b2_h,
            "woutt": woutt_h,
        })
    return in_maps


def kernel(x, W1, b1, W2, b2, Wout, bout, _trace=False, _trace_kwargs=None):
    x = np.asarray(x, dtype=np.float32)
    W1 = np.asarray(W1, dtype=np.float32)
    b1 = np.asarray(b1, dtype=np.float32)
    W2 = np.asarray(W2, dtype=np.float32)
    b2 = np.asarray(b2, dtype=np.float32)
    Wout = np.asarray(Wout, dtype=np.float32)
    bout = np.asarray(bout, dtype=np.float32)

    nc = get_program()
    in_maps = _prep_inputs(x, W1, b1, W2, b2, Wout, bout)
    res = run_bass_kernel_spmd(
        nc,
        in_maps,
        core_ids=list(range(N_CORES)),
        trace=_trace,
        **(_trace_kwargs or {}),
    )
    out = res.results[0]["out"].reshape(B, 1).astype(np.float32)
    if _trace:
        kernel.last_results = res
    return out


if __name__ == "__main__":
    # quick self-run with random data (not the reference distribution)
    rng = np.random.default_rng(0)
    x = (rng.random((B, IN_DIM)) < 32.0 / IN_DIM).astype(np.float32)
    W1 = rng.standard_normal((H1, IN_DIM), dtype=np.float32) / np.sqrt(IN_DIM)
    b1 = rng.standard_normal(H1, dtype=np.float32) / np.sqrt(IN_DIM)
    W2 = rng.standard_normal((H2, H1), dtype=np.float32) / np.sqrt(H1)
    b2 = rng.standard_normal(H2, dtype=np.float32) / np.sqrt(H1)
    Wout = rng.standard_normal((1, H2), dtype=np.float32) / np.sqrt(H2)
    bout = rng.standard_normal(1, dtype=np.float32) / np.sqrt(H2)
    got = kernel(x, W1, b1, W2, b2, Wout, bout)
    h1 = np.maximum(x @ W1.T + b1, 0)
    h2 = np.maximum(h1 @ W2.T + b2, 0)
    exp = h2 @ Wout.T + bout
    print("rel err:", np.abs(got - exp).max() / np.abs(exp).max())


# revision 12
# speedup vs baseline: 1.0292x; 1.0292x over previous
"""HalfKA NNUE forward pass on 8 Trainium2 NeuronCores.

Network (fp32 reference):
    h1  = relu(x @ W1.T + b1)     x:[2048, 98304] sparse 0/1, W1:[256, 98304]
    h2  = relu(h1 @ W2.T + b2)    W2:[32, 256]
    out = h2 @ Wout.T + bout      Wout:[1, 32]  -> [2048, 1]

Strategy: tensor-parallel over the fc1 contraction (input_dim). Each of the 8
cores reads its own 12288-column slice of x (the dominant 100 MB/core stream)
plus a 12288-row slice of W1, accumulates a partial h1 [256, 2048] on the PE
array, and the partials are summed with an on-device AllReduce. fc2/fc3 are
tiny and computed (redundantly) on every core in fp32.

fc1 runs as two bf16 passes (W1 split into bf16 hi + lo parts on the host,
accumulated in fp32 PSUM) which is fp32-accurate to ~1e-6 while streaming the
PE at 1 cycle/row instead of fp32's 4. x is 0/1 so its bf16 cast is exact.

The batch is processed in 4 chunks of 512 so each chunk's AllReduce overlaps
the next chunk's fc1 matmuls.
"""

import os
import sys

sys.path.insert(0, "/opt/trn_rl_repo")

from contextlib import ExitStack

import numpy as np
import ml_dtypes

import concourse.bass as bass
import concourse.tile as tile
from concourse import bacc, mybir
from concourse.bass_utils import run_bass_kernel_spmd

f32 = mybir.dt.float32
bf16 = mybir.dt.bfloat16

N_CORES = 8
B = 2048
IN_DIM = 98304
H1 = 256
H2 = 32

P = 128
KSH = IN_DIM // N_CORES          # 12288 contraction dims per core
KT = KSH // P                    # 96 k-tiles per core
SUP = 8                          # k-tiles per x DMA super-tile (16 KB/partition)
NSUP = KT // SUP                 # 12
CHUNK = 512                      # batch columns per chunk (one PSUM bank)
NCH = B // CHUNK                 # 4
M_T = H1 // P                    # 2 h1 partition-tiles

_CACHED = {}


def _build_program():
    nc = bacc.Bacc(
        "TRN2",
        target_bir_lowering=False,
        debug=False,
        num_devices=N_CORES,
    )

    xt = nc.dram_tensor("xt", [NCH, P, KT // SUP, SUP, CHUNK], f32, kind="ExternalInput")
    w1hi = nc.dram_tensor("w1hi", [P, KT, H1], bf16, kind="ExternalInput")
    w1lo = nc.dram_tensor("w1lo", [P, KT, H1], bf16, kind="ExternalInput")
    b1 = nc.dram_tensor("b1", [P, M_T], f32, kind="ExternalInput")
    w2t = nc.dram_tensor("w2t", [P, M_T, H2], f32, kind="ExternalInput")
    b2 = nc.dram_tensor("b2", [H2, 1], f32, kind="ExternalInput")
    # [Wout.T; bout] stacked: fc3 computes Wout @ h2 + bout via a ones-row in h2
    woutt = nc.dram_tensor("woutt", [H2 + 1, 1], f32, kind="ExternalInput")
    out = nc.dram_tensor("out", [NCH, CHUNK], f32, kind="ExternalOutput")

    with tile.TileContext(nc) as tc:
        with ExitStack() as ctx:
            const = ctx.enter_context(tc.tile_pool(name="const", bufs=1))
            xfp = ctx.enter_context(tc.tile_pool(name="xf", bufs=2))
            xbp = ctx.enter_context(tc.tile_pool(name="xb", bufs=2))
            drp = ctx.enter_context(tc.tile_pool(name="drain", bufs=4))
            h1rp = ctx.enter_context(tc.tile_pool(name="h1r", bufs=4))
            h1ap = ctx.enter_context(tc.tile_pool(name="h1a", bufs=4))
            smp = ctx.enter_context(tc.tile_pool(name="small", bufs=4))
            ps1 = ctx.enter_context(tc.tile_pool(name="ps1", bufs=2, space="PSUM"))
            ps2 = ctx.enter_context(tc.tile_pool(name="ps2", bufs=2, space="PSUM"))
            ps3 = ctx.enter_context(tc.tile_pool(name="ps3", bufs=2, space="PSUM"))
            dram = ctx.enter_context(tc.tile_pool(name="dram", bufs=2 * NCH, space="DRAM"))

            # ---- resident constants ----
            # W1 hi/lo stay resident all kernel, but are DMA'd in NSUP slices
            # interleaved with chunk 0's x loads so the PE can start without
            # waiting for the full 12.6 MB.
            w1hi_sl = [
                const.tile([P, SUP, H1], bf16, name=f"w1hi_sl{s}", tag=f"w1hi{s}")
                for s in range(NSUP)
            ]
            w1lo_sl = [
                const.tile([P, SUP, H1], bf16, name=f"w1lo_sl{s}", tag=f"w1lo{s}")
                for s in range(NSUP)
            ]
            b1_s = const.tile([P, M_T], f32)
            nc.sync.dma_start(b1_s[:], b1.ap())
            w2t_s = const.tile([P, M_T, H2], f32)
            nc.sync.dma_start(w2t_s[:], w2t.ap())
            b2_s = const.tile([H2, 1], f32)
            nc.sync.dma_start(b2_s[:], b2.ap())
            woutt_s = const.tile([H2 + 1, 1], f32)
            nc.sync.dma_start(woutt_s[:], woutt.ap())

            for j in range(NCH):
                # ---- fc1: partial h1[256, 512] for this chunk ----
                psum_m = [
                    ps1.tile([P, CHUNK], f32, tag=f"ps1_{m}", name=f"ps1m{m}_c{j}")
                    for m in range(M_T)
                ]
                for s in range(NSUP):
                    if j == 0:
                        nc.sync.dma_start(
                            w1hi_sl[s][:], w1hi.ap()[:, s * SUP:(s + 1) * SUP, :]
                        )
                        nc.sync.dma_start(
                            w1lo_sl[s][:], w1lo.ap()[:, s * SUP:(s + 1) * SUP, :]
                        )
                    xf = xfp.tile([P, SUP, CHUNK], f32)
                    nc.sync.dma_start(xf[:], xt.ap()[j, :, s, :, :])
                    xb = xbp.tile([P, SUP, CHUNK], bf16)
                    nc.vector.tensor_copy(xb[:], xf[:])
                    for tt in range(SUP):
                        t = s * SUP + tt
                        for m in range(M_T):
                            nc.tensor.matmul(
                                psum_m[m][:],
                                w1hi_sl[s][:, tt, m * P:(m + 1) * P],
                                xb[:, tt, :],
                                start=(t == 0),
                                stop=False,
                            )
                            nc.tensor.matmul(
                                psum_m[m][:],
                                w1lo_sl[s][:, tt, m * P:(m + 1) * P],
                                xb[:, tt, :],
                                start=False,
                                stop=(t == KT - 1),
                            )

                # ---- AllReduce the partial across the 8 cores ----
                cc_in = dram.tile([H1, CHUNK], f32, tag="cc_in")
                cc_out = dram.tile([H1, CHUNK], f32, tag="cc_out")
                for m in range(M_T):
                    dr = drp.tile([P, CHUNK], f32)
                    nc.vector.tensor_copy(dr[:], psum_m[m][:])
                    nc.sync.dma_start(cc_in[m * P:(m + 1) * P, :], dr[:])
                nc.gpsimd.collective_compute(
                    "AllReduce",
                    mybir.AluOpType.add,
                    replica_groups=[list(range(N_CORES))],
                    ins=[cc_in.opt()],
                    outs=[cc_out.opt()],
                )

                # ---- bias + relu -> h1, then fc2/fc3 in fp32 ----
                h1a = []
                for m in range(M_T):
                    h1r = h1rp.tile([P, CHUNK], f32)
                    nc.sync.dma_start(h1r[:], cc_out[m * P:(m + 1) * P, :])
                    act = h1ap.tile([P, CHUNK], f32)
                    nc.scalar.activation(
                        act[:], h1r[:],
                        mybir.ActivationFunctionType.Relu,
                        bias=b1_s[:, m:m + 1],
                    )
                    h1a.append(act)

                p2 = ps2.tile([H2, CHUNK], f32)
                for m in range(M_T):
                    nc.tensor.matmul(
                        p2[:], w2t_s[:, m, :], h1a[m][:],
                        start=(m == 0), stop=(m == M_T - 1),
                    )
                h2t = smp.tile([H2 + 1, CHUNK], f32, tag="h2")
                nc.scalar.activation(
                    h2t[0:H2, :], p2[:],
                    mybir.ActivationFunctionType.Relu,
                    bias=b2_s[:],
                )
                nc.vector.memset(h2t[H2:H2 + 1, :], 1.0)

                p3 = ps3.tile([1, CHUNK], f32)
                nc.tensor.matmul(p3[:], woutt_s[:], h2t[:], start=True, stop=True)
                ot = smp.tile([1, CHUNK], f32, tag="ot")
                nc.vector.tensor_copy(ot[:], p3[:])
                nc.sync.dma_start(out.ap()[j, :], ot[:])

    nc.compile()
    return nc


def get_program():
    if "nc" not in _CACHED:
        _CACHED["nc"] = _build_program()
    return _CACHED["nc"]


def _prep_inputs(x, W1, b1, W2, b2, Wout, bout):
    """Shard + lay out host-side into DMA-friendly per-core tensors."""
    bf = ml_dtypes.bfloat16

    # x: [2048, 98304] -> xT [98304, 2048] -> per core [NCH, P, NSUP, SUP, CHUNK]
    xT = np.ascontiguousarray(x.T)  # [98304, 2048]
    # x6[c] axes: [s, t, p, j, n]; device wants [j, p, s, t, n]
    x6 = xT.reshape(N_CORES, NSUP, SUP, P, NCH, CHUNK)
    in_maps = [
        {"xt": np.ascontiguousarray(x6[c].transpose(3, 2, 0, 1, 4))}
        for c in range(N_CORES)
    ]

    w1T = np.ascontiguousarray(W1.T)  # [98304, 256]
    b1_h = np.ascontiguousarray(b1.reshape(M_T, P).T)            # [P, M_T]
    w2t_h = np.ascontiguousarray(W2.T.reshape(M_T, P, H2).transpose(1, 0, 2))  # [P, M_T, H2]
    b2_h = np.ascontiguousarray(b2.reshape(H2, 1))
    woutt_h = np.concatenate(
        [Wout.T, bout.reshape(1, 1)], axis=0
    ).astype(np.float32)                                         # [H2+1, 1]

    for c in range(N_CORES):
        w1T_c = w1T[c * KSH:(c + 1) * KSH]                       # [12288, 256]
        hi = w1T_c.astype(bf)
        lo = (w1T_c - hi.astype(np.float32)).astype(bf)
        # [KSH, H1] -> [P, KT, H1]: row (t*P + p) -> [p, t]
        hi = np.ascontiguousarray(hi.reshape(KT, P, H1).transpose(1, 0, 2))
        lo = np.ascontiguousarray(lo.reshape(KT, P, H1).transpose(1, 0, 2))
        in_maps[c].update({
            "w1hi": hi,
            "w1lo": lo,
            "b1": b1_h,
            "w2t": w2t_h,
            "b2": b2_h,
            "woutt": woutt_h,
        })
    return in_maps


def kernel(x, W1, b1, W2, b2, Wout, bout, _trace=False, _trace_kwargs=None):
    x = np.asarray(x, dtype=np.float32)
    W1 = np.asarray(W1, dtype=np.float32)
    b1 = np.asarray(b1, dtype=np.float32)
    W2 = np.asarray(W2, dtype=np.float32)
    b2 = np.asarray(b2, dtype=np.float32)
    Wout = np.asarray(Wout, dtype=np.float32)
    bout = np.asarray(bout, dtype=np.float32)

    nc = get_program()
    in_maps = _prep_inputs(x, W1, b1, W2, b2, Wout, bout)
    res = run_bass_kernel_spmd(
        nc,
        in_maps,
        core_ids=list(range(N_CORES)),
        trace=_trace,
        **(_trace_kwargs or {}),
    )
    out = res.results[0]["out"].reshape(B, 1).astype(np.float32)
    if _trace:
        kernel.last_results = res
    return out


if __name__ == "__main__":
    # quick self-run with random data (not the reference distribution)
    rng = np.random.default_rng(0)
    x = (rng.random((B, IN_DIM)) < 32.0 / IN_DIM).astype(np.float32)
    W1 = rng.standard_normal((H1, IN_DIM), dtype=np.float32) / np.sqrt(IN_DIM)
    b1 = rng.standard_normal(H1, dtype=np.float32) / np.sqrt(IN_DIM)
    W2 = rng.standard_normal((H2, H1), dtype=np.float32) / np.sqrt(H1)
    b2 = rng.standard_normal(H2, dtype=np.float32) / np.sqrt(H1)
    Wout = rng.standard_normal((1, H2), dtype=np.float32) / np.sqrt(H2)
    bout = rng.standard_normal(1, dtype=np.float32) / np.sqrt(H2)
    got = kernel(x, W1, b1, W2, b2, Wout, bout)
    h1 = np.maximum(x @ W1.T + b1, 0)
    h2 = np.maximum(h1 @ W2.T + b2, 0)
    exp = h2 @ Wout.T + bout
    print("rel err:", np.abs(got - exp).max() / np.abs(exp).max())


# revision 25
# speedup vs baseline: 1.2008x; 1.1667x over previous
"""HalfKA NNUE forward pass on 8 Trainium2 NeuronCores.

Network (fp32 reference):
    h1  = relu(x @ W1.T + b1)     x:[2048, 98304] sparse 0/1, W1:[256, 98304]
    h2  = relu(h1 @ W2.T + b2)    W2:[32, 256]
    out = h2 @ Wout.T + bout      Wout:[1, 32]  -> [2048, 1]

Strategy: tensor-parallel over the fc1 contraction (input_dim). Each of the 8
cores reads its own 12288-column slice of x (the dominant 100 MB/core stream)
plus a 12288-row slice of W1, accumulates a partial h1 [256, 2048] on the PE
array, and the partials are summed with an on-device AllReduce. fc2/fc3 are
tiny and computed (redundantly) on every core in fp32.

fc1 runs as two bf16 passes (W1 split into bf16 hi + lo parts on the host,
accumulated in fp32 PSUM) which is fp32-accurate to ~1e-6 while streaming the
PE at 1 cycle/row instead of fp32's 4. x is 0/1 so its bf16 cast is exact.

The batch is processed in 4 chunks of 512 so each chunk's AllReduce overlaps
the next chunk's fc1 matmuls.
"""

import os
import sys

sys.path.insert(0, "/opt/trn_rl_repo")

from contextlib import ExitStack

import numpy as np
import ml_dtypes

import concourse.bass as bass
import concourse.tile as tile
from concourse import bacc, mybir
from concourse.bass_utils import run_bass_kernel_spmd

f32 = mybir.dt.float32
bf16 = mybir.dt.bfloat16

N_CORES = 8
B = 2048
IN_DIM = 98304
H1 = 256
H2 = 32

P = 128
# fc1 weight passes: 1 = bf16 (rel err ~7e-4), 2 = bf16 hi+lo (rel err ~1e-6)
W1_PASSES = 1
KSH = IN_DIM // N_CORES          # 12288 contraction dims per core
KT = KSH // P                    # 96 k-tiles per core
SUP = 8                          # k-tiles per x DMA super-tile (16 KB/partition)
NSUP = KT // SUP                 # 12
CHUNK = 512                      # batch columns per chunk (one PSUM bank)
NCH = B // CHUNK                 # 4
M_T = H1 // P                    # 2 h1 partition-tiles

_CACHED = {}


def _build_program():
    nc = bacc.Bacc(
        "TRN2",
        target_bir_lowering=False,
        debug=False,
        num_devices=N_CORES,
    )

    xt = nc.dram_tensor("xt", [NCH, P, KT // SUP, SUP, CHUNK], f32, kind="ExternalInput")
    w1hi = nc.dram_tensor("w1hi", [P, KT, H1], bf16, kind="ExternalInput")
    w1lo = (
        nc.dram_tensor("w1lo", [P, KT, H1], bf16, kind="ExternalInput")
        if W1_PASSES == 2 else None
    )
    b1 = nc.dram_tensor("b1", [P, M_T], f32, kind="ExternalInput")
    w2t = nc.dram_tensor("w2t", [P, M_T, H2], f32, kind="ExternalInput")
    b2 = nc.dram_tensor("b2", [H2, 1], f32, kind="ExternalInput")
    # [Wout.T; bout] stacked: fc3 computes Wout @ h2 + bout via a ones-row in h2
    woutt = nc.dram_tensor("woutt", [H2 + 1, 1], f32, kind="ExternalInput")
    out = nc.dram_tensor("out", [NCH, CHUNK], f32, kind="ExternalOutput")

    with tile.TileContext(nc) as tc:
        with ExitStack() as ctx:
            const = ctx.enter_context(tc.tile_pool(name="const", bufs=1))
            xfp = ctx.enter_context(tc.tile_pool(name="xf", bufs=5))
            xbp = ctx.enter_context(tc.tile_pool(name="xb", bufs=3))
            drp = ctx.enter_context(tc.tile_pool(name="drain", bufs=4))
            h1rp = ctx.enter_context(tc.tile_pool(name="h1r", bufs=4))
            h1ap = ctx.enter_context(tc.tile_pool(name="h1a", bufs=4))
            smp = ctx.enter_context(tc.tile_pool(name="small", bufs=4))
            ps1 = ctx.enter_context(tc.tile_pool(name="ps1", bufs=2, space="PSUM"))
            ps2 = ctx.enter_context(tc.tile_pool(name="ps2", bufs=2, space="PSUM"))
            ps3 = ctx.enter_context(tc.tile_pool(name="ps3", bufs=2, space="PSUM"))
            dram = ctx.enter_context(tc.tile_pool(name="dram", bufs=2 * NCH, space="DRAM"))

            # ---- resident constants ----
            # W1 hi/lo stay resident all kernel, but are DMA'd in NSUP slices
            # interleaved with chunk 0's x loads so the PE can start without
            # waiting for the full 12.6 MB.
            w1hi_sl = [
                const.tile([P, SUP, H1], bf16, name=f"w1hi_sl{s}", tag=f"w1hi{s}")
                for s in range(NSUP)
            ]
            w1lo_sl = [
                const.tile([P, SUP, H1], bf16, name=f"w1lo_sl{s}", tag=f"w1lo{s}")
                for s in range(NSUP)
            ] if W1_PASSES == 2 else None
            b1_s = const.tile([P, M_T], f32)
            nc.sync.dma_start(b1_s[:], b1.ap())
            w2t_s = const.tile([P, M_T, H2], f32)
            nc.sync.dma_start(w2t_s[:], w2t.ap())
            b2_s = const.tile([H2, 1], f32)
            nc.sync.dma_start(b2_s[:], b2.ap())
            woutt_s = const.tile([H2 + 1, 1], f32)
            nc.sync.dma_start(woutt_s[:], woutt.ap())

            # ---- phase 2: bias+relu, fc2, fc3 for one chunk ----
            def phase2(j):
                cc_out = cc_outs[j]
                h1a = []
                for m in range(M_T):
                    h1r = h1rp.tile([P, CHUNK], f32, name=f"h1r{j}_{m}", tag="h1r")
                    nc.sync.dma_start(h1r[:], cc_out[m * P:(m + 1) * P, :])
                    act = h1ap.tile([P, CHUNK], f32, name=f"act{j}_{m}", tag="act")
                    nc.scalar.activation(
                        act[:], h1r[:],
                        mybir.ActivationFunctionType.Relu,
                        bias=b1_s[:, m:m + 1],
                    )
                    h1a.append(act)

                p2 = ps2.tile([H2, CHUNK], f32, name=f"p2_{j}", tag="p2")
                for m in range(M_T):
                    nc.tensor.matmul(
                        p2[:], w2t_s[:, m, :], h1a[m][:],
                        start=(m == 0), stop=(m == M_T - 1),
                    )
                h2t = smp.tile([H2 + 1, CHUNK], f32, tag="h2", name=f"h2t{j}")
                nc.scalar.activation(
                    h2t[0:H2, :], p2[:],
                    mybir.ActivationFunctionType.Relu,
                    bias=b2_s[:],
                )
                nc.vector.memset(h2t[H2:H2 + 1, :], 1.0)

                p3 = ps3.tile([1, CHUNK], f32, name=f"p3_{j}", tag="p3")
                nc.tensor.matmul(p3[:], woutt_s[:], h2t[:], start=True, stop=True)
                ot = smp.tile([1, CHUNK], f32, tag="ot", name=f"ot{j}")
                nc.vector.tensor_copy(ot[:], p3[:])
                nc.sync.dma_start(out.ap()[j, :], ot[:])

            # ---- phase 1: fc1 for all chunks; each chunk's partial goes
            # straight into its AllReduce so the collectives overlap the
            # next chunk's matmuls. Chunk j-1's fc2/fc3 are emitted after
            # chunk j's fc1 so the PE never idle-waits on an AllReduce
            # except for the final chunk's.
            cc_outs = []
            _loaded_w1 = set()
            for j in range(NCH):
                psum_m = [
                    ps1.tile([P, CHUNK], f32, tag=f"ps1_{m}", name=f"ps1m{m}_c{j}")
                    for m in range(M_T)
                ]
                # first super-tile of the kernel is small so the PE starts early
                sched = [2, 6] + [SUP] * (NSUP - 1) if j == 0 else [SUP] * NSUP
                s_off = 0  # k-tile offset
                for si, sup in enumerate(sched):
                    if j == 0:
                        # W1 slices are aligned to the fixed SUP grid
                        for s in range(s_off // SUP, (s_off + sup + SUP - 1) // SUP):
                            if s < NSUP and s not in _loaded_w1:
                                _loaded_w1.add(s)
                                nc.sync.dma_start(
                                    w1hi_sl[s][:], w1hi.ap()[:, s * SUP:(s + 1) * SUP, :]
                                )
                                if W1_PASSES == 2:
                                    nc.sync.dma_start(
                                        w1lo_sl[s][:], w1lo.ap()[:, s * SUP:(s + 1) * SUP, :]
                                    )
                    xf = xfp.tile([P, SUP, CHUNK], f32, name=f"xf_{j}_{si}", tag="xf")
                    nc.sync.dma_start(
                        xf[:, 0:sup, :],
                        xt.ap()[j, :, :, :, :].rearrange("p s t n -> p (s t) n")[
                            :, s_off:s_off + sup, :
                        ],
                    )
                    xb = xbp.tile([P, SUP, CHUNK], bf16, name=f"xb_{j}_{si}", tag="xb")
                    nc.vector.tensor_copy(xb[:, 0:sup, :], xf[:, 0:sup, :])
                    for tt in range(sup):
                        t = s_off + tt
                        s, ts = divmod(t, SUP)
                        for m in range(M_T):
                            nc.tensor.matmul(
                                psum_m[m][:],
                                w1hi_sl[s][:, ts, m * P:(m + 1) * P],
                                xb[:, tt, :],
                                start=(t == 0),
                                stop=(W1_PASSES == 1 and t == KT - 1),
                            )
                            if W1_PASSES == 2:
                                nc.tensor.matmul(
                                    psum_m[m][:],
                                    w1lo_sl[s][:, ts, m * P:(m + 1) * P],
                                    xb[:, tt, :],
                                    start=False,
                                    stop=(t == KT - 1),
                                )
                    s_off += sup

                cc_in = dram.tile([H1, CHUNK], f32, tag="cc_in", name=f"cc_in{j}")
                cc_out = dram.tile([H1, CHUNK], f32, tag="cc_out", name=f"cc_out{j}")
                for m in range(M_T):
                    dr = drp.tile([P, CHUNK], f32, name=f"dr{j}_{m}", tag="dr")
                    nc.vector.tensor_copy(dr[:], psum_m[m][:])
                    nc.sync.dma_start(cc_in[m * P:(m + 1) * P, :], dr[:])
                nc.gpsimd.collective_compute(
                    "AllReduce",
                    mybir.AluOpType.add,
                    replica_groups=[list(range(N_CORES))],
                    ins=[cc_in.opt()],
                    outs=[cc_out.opt()],
                )
                cc_outs.append(cc_out)
                if j > 0:
                    phase2(j - 1)
            phase2(NCH - 1)

    nc.compile()
    return nc


def get_program():
    if "nc" not in _CACHED:
        _CACHED["nc"] = _build_program()
    return _CACHED["nc"]


def _prep_inputs(x, W1, b1, W2, b2, Wout, bout):
    """Shard + lay out host-side into DMA-friendly per-core tensors."""
    bf = ml_dtypes.bfloat16

    # x: [2048, 98304] -> xT [98304, 2048] -> per core [NCH, P, NSUP, SUP, CHUNK]
    xT = np.ascontiguousarray(x.T)  # [98304, 2048]
    # x6[c] axes: [s, t, p, j, n]; device wants [j, p, s, t, n]
    x6 = xT.reshape(N_CORES, NSUP, SUP, P, NCH, CHUNK)
    in_maps = [
        {"xt": np.ascontiguousarray(x6[c].transpose(3, 2, 0, 1, 4))}
        for c in range(N_CORES)
    ]

    w1T = np.ascontiguousarray(W1.T)  # [98304, 256]
    b1_h = np.ascontiguousarray(b1.reshape(M_T, P).T)            # [P, M_T]
    w2t_h = np.ascontiguousarray(W2.T.reshape(M_T, P, H2).transpose(1, 0, 2))  # [P, M_T, H2]
    b2_h = np.ascontiguousarray(b2.reshape(H2, 1))
    woutt_h = np.concatenate(
        [Wout.T, bout.reshape(1, 1)], axis=0
    ).astype(np.float32)                                         # [H2+1, 1]

    for c in range(N_CORES):
        w1T_c = w1T[c * KSH:(c + 1) * KSH]                       # [12288, 256]
        hi = w1T_c.astype(bf)
        # [KSH, H1] -> [P, KT, H1]: row (t*P + p) -> [p, t]
        hi = np.ascontiguousarray(hi.reshape(KT, P, H1).transpose(1, 0, 2))
        in_maps[c].update({
            "w1hi": hi,
            "b1": b1_h,
            "w2t": w2t_h,
            "b2": b2_h,
            "woutt": woutt_h,
        })
        if W1_PASSES == 2:
            lo = (w1T_c - in_maps[c]["w1hi"].astype(np.float32).transpose(1, 0, 2).reshape(KSH, H1)).astype(bf)
            in_maps[c]["w1lo"] = np.ascontiguousarray(lo.reshape(KT, P, H1).transpose(1, 0, 2))
    return in_maps


def kernel(x, W1, b1, W2, b2, Wout, bout, _trace=False, _trace_kwargs=None):
    x = np.asarray(x, dtype=np.float32)
    W1 = np.asarray(W1, dtype=np.float32)
    b1 = np.asarray(b1, dtype=np.float32)
    W2 = np.asarray(W2, dtype=np.float32)
    b2 = np.asarray(b2, dtype=np.float32)
    Wout = np.asarray(Wout, dtype=np.float32)
    bout = np.asarray(bout, dtype=np.float32)

    nc = get_program()
    in_maps = _prep_inputs(x, W1, b1, W2, b2, Wout, bout)
    res = run_bass_kernel_spmd(
        nc,
        in_maps,
        core_ids=list(range(N_CORES)),
        trace=_trace,
        **(_trace_kwargs or {}),
    )
    out = res.results[0]["out"].reshape(B, 1).astype(np.float32)
    if _trace:
        kernel.last_results = res
    return out


if __name__ == "__main__":
    # quick self-run with random data (not the reference distribution)
    rng = np.random.default_rng(0)
    x = (rng.random((B, IN_DIM)) < 32.0 / IN_DIM).astype(np.float32)
    W1 = rng.standard_normal((H1, IN_DIM), dtype=np.float32) / np.sqrt(IN_DIM)
    b1 = rng.standard_normal(H1, dtype=np.float32) / np.sqrt(IN_DIM)
    W2 = rng.standard_normal((H2, H1), dtype=np.float32) / np.sqrt(H1)
    b2 = rng.standard_normal(H2, dtype=np.float32) / np.sqrt(H1)
    Wout = rng.standard_normal((1, H2), dtype=np.float32) / np.sqrt(H2)
    bout = rng.standard_normal(1, dtype=np.float32) / np.sqrt(H2)
    got = kernel(x, W1, b1, W2, b2, Wout, bout)
    h1 = np.maximum(x @ W1.T + b1, 0)
    h2 = np.maximum(h1 @ W2.T + b2, 0)
    exp = h2 @ Wout.T + bout
    print("rel err:", np.abs(got - exp).max() / np.abs(exp).max())
